# revision 10
# baseline (speedup 1.0000x reference)
"""Trainium2 Bass kernel for a 2-layer DPHGNN + hyperconv GNN message-passing stack.

kernel(**inputs) takes FULL numpy inputs (as produced by the problem's
setup_inputs()) and returns the FULL [50000, 128] float32 output.

Distribution: nodes (and their incident hypergraph entries) are sharded across
8 NeuronCores; edge-side tensors are exchanged with ReduceScatter / AllGather.
Gathers use the custom dma_gather SWDGE ucode; segment sums run on the tensor
engine via one-hot selection-matrix matmuls accumulated in PSUM.
"""

import sys
from contextlib import ExitStack

for _p in ("/opt/trn_rl_repo",):
    if _p not in sys.path:
        sys.path.append(_p)

import numpy as np

import concourse.bass as bass
import concourse.bacc as bacc
import concourse.mybir as mybir
import concourse.tile as tile
from concourse.bass_utils import run_bass_kernel_spmd
from concourse.masks import make_identity

F32 = mybir.dt.float32
BF16 = mybir.dt.bfloat16
I16 = mybir.dt.int16
AF = mybir.ActivationFunctionType

NEG_SLOPE = 0.2
P = 128
NCORES = 8
GQ = 4          # SWDGE queues
NI = 1024       # rows per dma_gather call (8 chunks of 128)
WCH = 8         # chunks per gather call / A-build batch
PSW = 2         # PSUM tiles per scatter mega-window


def _wrap_idx(flat):
    """[L] -> [128, L/16] int16: wrapped in 16 partitions, replicated x8."""
    L = len(flat)
    assert L % 16 == 0
    blk = np.asarray(flat, np.int16).reshape(-1, 16).T.copy()
    return np.ascontiguousarray(np.tile(blk, (8, 1)))


def _build_stream(dst, src_idx, n_tiles, cpt):
    """Destination-sorted, per-tile 128-padded entry stream.
    Returns (gidx[L], ec[L]) with ec = dst - tile*128 (pads: gidx 0, ec -1)."""
    order = np.argsort(dst, kind="stable")
    dsts = np.asarray(dst)[order]
    srcs = np.asarray(src_idx)[order]
    tile_of = dsts // P
    counts = np.bincount(tile_of, minlength=n_tiles)
    base = np.concatenate([[0], np.cumsum(cpt * P)])
    L = int(base[-1])
    gidx = np.zeros(L, np.int64)
    ec = -np.ones(L, np.float32)
    starts = np.concatenate([[0], np.cumsum(counts)])
    off = np.arange(len(dsts)) - starts[tile_of]
    slot = base[tile_of] + off
    gidx[slot] = srcs
    ec[slot] = dsts - tile_of * P
    return gidx, ec


def _prep(inputs):
    N, M = 50000, 20000
    NS, MS = N // NCORES, M // NCORES
    V = np.asarray(inputs["V"]).astype(np.int64)
    E = np.asarray(inputs["E"]).astype(np.int64)
    X = np.asarray(inputs["X"], np.float32)
    S = np.asarray(inputs["S"], np.float32)

    NT_E = (M + P - 1) // P           # 157
    NT_V = (NS + P - 1) // P          # 49
    NT_MS = (MS + P - 1) // P         # 20

    # graph-structure constants (mirroring the reference semantics)
    deg_v = np.bincount(V, minlength=N).astype(np.float64)
    cnt_e = np.bincount(E, minlength=M).astype(np.float64)
    deginv = np.where(deg_v > 0, 1.0 / np.maximum(deg_v, 1.0), 0.0)
    De = np.zeros(M, np.float64)
    np.add.at(De, E, deg_v[V])
    De = De / (cnt_e + 1.0)
    De_inv = np.where(De > 0, De ** -0.5, 1.0)
    coef_e = np.where(cnt_e > 0, De_inv / np.maximum(cnt_e, 1.0), De_inv * 0.0)
    Dv_inv = np.where(deg_v > 0, deg_v ** -0.5, 0.0)

    owner = V // NS
    v2e_raw, e2v_raw = [], []
    for c in range(NCORES):
        m = owner == c
        Vl = V[m] - c * NS
        Ee = E[m]
        v2e_raw.append((Ee, Vl))
        e2v_raw.append((Vl, Ee))

    def caps(raw, n_tiles):
        cpt = None
        for dst, _ in raw:
            counts = np.bincount(np.asarray(dst) // P, minlength=n_tiles)
            c1 = np.maximum(1, (counts + P - 1) // P)
            cpt = c1 if cpt is None else np.maximum(cpt, c1)
        return cpt

    cpt_v2e = caps(v2e_raw, NT_E)
    cpt_e2v = caps(e2v_raw, NT_V)
    Lv = int(np.sum(cpt_v2e) * P)
    Le = int(np.sum(cpt_e2v) * P)
    LvP = ((Lv + NI - 1) // NI) * NI
    LeP = ((Le + NI - 1) // NI) * NI

    def pad_stream(g, ec, LP):
        gi = np.full(LP, -1, np.int64)
        gi[: len(g)] = g
        ecp = np.full(LP, -1.0, np.float32)
        ecp[: len(ec)] = ec
        return gi, np.ascontiguousarray(ecp.reshape(-1, P).T)

    cores = []
    for c in range(NCORES):
        gv, ecv = _build_stream(*v2e_raw[c], NT_E, cpt_v2e)
        ge, ece = _build_stream(*e2v_raw[c], NT_V, cpt_e2v)
        gv_p, ecv_2d = pad_stream(gv, ecv, LvP)
        ge_p, ece_2d = pad_stream(ge, ece, LeP)
        cores.append(dict(gv_p=gv_p, ecv_2d=ecv_2d, ge_p=ge_p, ece_2d=ece_2d))

    def regs(L, LP):
        return [int(max(0, min(L - k * NI, NI))) for k in range(LP // NI)]

    regs_v, regs_e = regs(Lv, LvP), regs(Le, LeP)

    # folded weights
    g = lambda k: np.asarray(inputs[k], np.float32)
    W = {}
    for l in range(2):
        Wv, bv, a = g(f"Wv{l}"), g(f"bv{l}"), g(f"a{l}")
        Wx, bx = g(f"Wx{l}"), g(f"bx{l}")
        Wt, bt = g(f"Wt{l}"), g(f"bt{l}")
        Wva = np.concatenate([Wv, (Wv @ a)[:, None]], axis=1)
        bva = np.concatenate([bv, [float(bv @ a)]])
        Wt_top, Wt_bot = Wt[:256], Wt[256:]
        btf = bt - Wt_top.sum(axis=0)
        nh = Wva.shape[0] // P           # 1 (layer0) or 2 (layer1)
        for hi in range(nh):
            W[f"Wva{l}h{hi}"] = np.ascontiguousarray(Wva[hi * P:(hi + 1) * P])
            W[f"Wx{l}h{hi}"] = np.ascontiguousarray(Wx[hi * P:(hi + 1) * P])
        W[f"bva{l}"] = np.tile(bva[None, :].astype(np.float32), (P, 1))
        W[f"bx{l}"] = np.tile((bx - 1.0)[None, :], (P, 1))
        W[f"Wt{l}h0"] = np.ascontiguousarray(Wt_top[:128])
        W[f"Wt{l}h1"] = np.ascontiguousarray(Wt_top[128:])
        W[f"Wt{l}bot"] = np.ascontiguousarray(Wt_bot)
        W[f"bt{l}"] = np.tile(btf[None, :].astype(np.float32), (P, 1))
    Wf = g("Wf")
    W["Wfh0"] = np.ascontiguousarray(Wf[:128])
    W["Wfh1"] = np.ascontiguousarray(Wf[128:])
    W["bf"] = np.tile(g("bf")[None, :], (P, 1))

    iota = np.tile(np.arange(P, dtype=np.float32)[None, :], (P, 1))
    iota_rep = np.ascontiguousarray(
        np.broadcast_to(iota[:, None, :], (P, WCH, P)))

    def cols(arr_shard, n_tiles):
        out = np.zeros((P, n_tiles), np.float32)
        a = np.asarray(arr_shard, np.float32)
        for t in range(n_tiles):
            seg = a[t * P:(t + 1) * P]
            out[: len(seg), t] = seg
        return out

    in_maps = []
    for c in range(NCORES):
        d = cores[c]
        im = dict(
            XT=np.ascontiguousarray(X[c * NS:(c + 1) * NS].T),
            ST=np.ascontiguousarray(S[c * MS:(c + 1) * MS].T),
            gv_idx=_wrap_idx(d["gv_p"]), ge_idx=_wrap_idx(d["ge_p"]),
            ec_v=d["ecv_2d"], ec_e=d["ece_2d"],
            iota_rep=iota_rep,
            deginv_c=cols(deginv[c * NS:(c + 1) * NS], NT_V),
            dvinv_c=cols(Dv_inv[c * NS:(c + 1) * NS], NT_V),
            coef_c=cols(coef_e[c * MS:(c + 1) * MS], NT_MS),
        )
        im.update(W)
        in_maps.append(im)

    meta = dict(N=N, M=M, NS=NS, MS=MS, NT_E=NT_E, NT_V=NT_V, NT_MS=NT_MS,
                cpt_v2e=[int(x) for x in cpt_v2e],
                cpt_e2v=[int(x) for x in cpt_e2v],
                LvP=LvP, LeP=LeP, regs_v=regs_v, regs_e=regs_e)
    return in_maps, meta


# ---------------------------------------------------------------------------

def build_program(meta):
    NS, MS, M = meta["NS"], meta["MS"], meta["M"]
    NT_E, NT_V, NT_MS = meta["NT_E"], meta["NT_V"], meta["NT_MS"]
    ESV, ESE, ESH = 384, 256, 128

    nc = bacc.Bacc("TRN2", target_bir_lowering=False, debug=False,
                   num_devices=NCORES, num_swdge_queues=GQ)

    def din(name, shape, dt=F32):
        return nc.dram_tensor(name, shape, dt, kind="ExternalInput")

    XT = din("XT", [P, NS])
    ST = din("ST", [64, MS])
    gv_idx = din("gv_idx", [P, meta["LvP"] // 16], I16)
    ge_idx = din("ge_idx", [P, meta["LeP"] // 16], I16)
    nch_v = meta["LvP"] // P
    nch_e = meta["LeP"] // P
    ec_v = din("ec_v", [P, nch_v])
    ec_e = din("ec_e", [P, nch_e])
    iota_rep = din("iota_rep", [P, WCH, P])
    deginv_c = din("deginv_c", [P, NT_V])
    dvinv_c = din("dvinv_c", [P, NT_V])
    coef_c = din("coef_c", [P, NT_MS])
    wnames = (["Wva0h0", "Wx0h0", "Wva1h0", "Wva1h1", "Wx1h0", "Wx1h1",
               "Wt0h0", "Wt0h1", "Wt1h0", "Wt1h1", "Wfh0", "Wfh1"],
              ["bva0", "bx0", "bva1", "bx1", "bt0", "bt1", "bf"],
              ["Wt0bot", "Wt1bot"])
    wshapes = dict(Wva0h0=[P, 257], Wx0h0=[P, 256],
                   Wva1h0=[P, 257], Wva1h1=[P, 257],
                   Wx1h0=[P, 256], Wx1h1=[P, 256],
                   Wt0h0=[P, 256], Wt0h1=[P, 256],
                   Wt1h0=[P, 256], Wt1h1=[P, 256],
                   Wfh0=[P, 128], Wfh1=[P, 128],
                   bva0=[P, 257], bx0=[P, 256], bva1=[P, 257], bx1=[P, 256],
                   bt0=[P, 256], bt1=[P, 256], bf=[P, 128],
                   Wt0bot=[64, 256], Wt1bot=[64, 256])
    Wd = {k: din(k, wshapes[k]) for grp in wnames for k in grp}

    yout = nc.dram_tensor("yout", [NS, 128], F32, kind="ExternalOutput")

    with tile.TileContext(nc) as tc:
        ctx = ExitStack()
        sbuf = ctx.enter_context(tc.tile_pool(name="sbuf", bufs=2))
        psum = ctx.enter_context(tc.tile_pool(name="psum", bufs=2, space="PSUM"))
        dram = ctx.enter_context(tc.tile_pool(name="dram", bufs=1, space="DRAM"))
        cons = ctx.enter_context(tc.tile_pool(name="cons", bufs=1))

        iota_t = cons.tile([P, WCH, P], F32, name="iota_t")
        nc.sync.dma_start(iota_t[:], iota_rep[:])
        ident = cons.tile([P, P], F32, name="ident")
        make_identity(nc, ident[:])
        wt = {}
        for k, h in Wd.items():
            t = cons.tile(list(h.shape), F32, name=f"w_{k}")
            nc.sync.dma_start(t[:], h[:])
            wt[k] = t
        st_t = cons.tile([64, MS], F32, name="st_t")
        nc.sync.dma_start(st_t[:], ST[:])
        ecv_t = cons.tile([P, nch_v], F32, name="ecv_t")
        nc.sync.dma_start(ecv_t[:], ec_v[:])
        ece_t = cons.tile([P, nch_e], F32, name="ece_t")
        nc.sync.dma_start(ece_t[:], ec_e[:])
        gvi_t = cons.tile([P, meta["LvP"] // 16], I16, name="gvi_t")
        nc.sync.dma_start(gvi_t[:], gv_idx[:])
        gei_t = cons.tile([P, meta["LeP"] // 16], I16, name="gei_t")
        nc.sync.dma_start(gei_t[:], ge_idx[:])
        dgi_t = cons.tile([P, NT_V], F32, name="dgi_t")
        nc.sync.dma_start(dgi_t[:], deginv_c[:])
        dvi_t = cons.tile([P, NT_V], F32, name="dvi_t")
        nc.sync.dma_start(dvi_t[:], dvinv_c[:])
        cf_t = cons.tile([P, NT_MS], F32, name="cf_t")
        nc.sync.dma_start(cf_t[:], coef_c[:])

        hT = [dram.tile([P, NS], F32, name="hT0"),
              dram.tile([P, NS], F32, name="hT1")]

        def scatter_pass(table, es, used_cols, idx_t, ec_t, cpt, n_tiles,
                         regs, out_dram, out_cols, tag):
            tile_of_chunk = []
            for t, n in enumerate(cpt):
                tile_of_chunk += [t] * n
            n_chunks = len(tile_of_chunk)
            first_c, last_c = {}, {}
            for k, t in enumerate(tile_of_chunk):
                first_c.setdefault(t, k)
                last_c[t] = k
            g_tiles = []
            for call in range(len(regs)):
                if regs[call] == 0:
                    g_tiles.append(None)
                    continue
                gt = sbuf.tile([P, WCH, es], BF16, tag="g", bufs=8,
                               name=f"g{tag}_{call}")
                nc.gpsimd.dma_gather(
                    out_ap=gt[:], in_ap=table[:],
                    idxs_ap=idx_t[:, call * (NI // 16):(call + 1) * (NI // 16)],
                    num_idxs=NI, num_idxs_reg=regs[call], elem_size=es,
                    queue_num=call % GQ)
                g_tiles.append(gt)
            a_cur = [None, -1]
            mega = [None, -1]   # tile, window index
            stg_cur = [None]
            for k in range(n_chunks):
                t = tile_of_chunk[k]
                call, j = k // WCH, k % WCH
                gt = g_tiles[call]
                if gt is None:
                    continue
                w = k // WCH
                if a_cur[1] != w:
                    ab = sbuf.tile([P, WCH, P], BF16, tag="A", bufs=6,
                                   name=f"A{tag}_{w}")
                    nc.vector.tensor_tensor(
                        out=ab[:],
                        in0=ec_t[:, w * WCH:(w + 1) * WCH].to_broadcast(
                            [P, WCH, P]),
                        in1=iota_t[:],
                        op=mybir.AluOpType.is_equal)
                    a_cur = [ab, w]
                mw = t // PSW
                if mega[1] != mw:
                    mega = [psum.tile([P, PSW, 512], F32, tag="ps", bufs=3,
                                      name=f"ps{tag}_{mw}"), mw]
                pt = mega[0]
                q = t % PSW
                nc.tensor.matmul(
                    out=pt[:, q, 0:out_cols],
                    lhsT=a_cur[0][:, j, :],
                    rhs=gt[:, j, 0:used_cols],
                    start=(k == first_c[t]), stop=(k == last_c[t]))
                if k == last_c[t]:
                    if stg_cur[0] is None:
                        stg_cur[0] = sbuf.tile([P, PSW, out_cols],
                                               out_dram.dtype,
                                               tag="pstg", bufs=4,
                                               name=f"pstg{tag}_{t}")
                    nc.vector.tensor_copy(out=stg_cur[0][:, q, :],
                                          in_=pt[:, q, 0:out_cols])
                    if t % PSW == PSW - 1 or t == n_tiles - 1:
                        n_in_w = q + 1
                        rows0 = (t - n_in_w + 1) * P
                        nc.sync.dma_start(
                            out=out_dram[rows0:rows0 + n_in_w * P, :].rearrange(
                                "(j p) c -> p j c", p=P),
                            in_=stg_cur[0][:, 0:n_in_w, :])
                        stg_cur[0] = None

        def elu_u(z_ap, w, cols, tag, i):
            mn = sbuf.tile([P, cols], F32, tag="mn", bufs=2, name=f"mn{tag}{i}")
            nc.vector.tensor_scalar_min(out=mn[:w], in0=z_ap, scalar1=0.0)
            ex = sbuf.tile([P, cols], F32, tag="ex", bufs=2, name=f"ex{tag}{i}")
            nc.scalar.activation(ex[:w], mn[:w], AF.Exp)
            rl = sbuf.tile([P, cols], F32, tag="rl", bufs=2, name=f"rl{tag}{i}")
            nc.scalar.activation(rl[:w], z_ap, AF.Relu)
            u = sbuf.tile([P, cols], F32, tag="u", bufs=2, name=f"u{tag}{i}")
            nc.vector.tensor_add(u[:w], rl[:w], ex[:w])
            return u

        def dense_and_table(l, lhsT_of, table, xinit_dram):
            nh = 1 if l == 0 else 2
            for t in range(NT_V):
                w = min(P, NS - t * P)
                halves = lhsT_of(t, w)
                pf = psum.tile([P, 512], F32, tag="pd", bufs=1, name=f"pf{l}_{t}")
                for hi in range(nh):
                    nc.tensor.matmul(out=pf[:w, 0:257], lhsT=halves[hi],
                                     rhs=wt[f"Wva{l}h{hi}"][:],
                                     start=(hi == 0), stop=(hi == nh - 1))
                F = sbuf.tile([P, 257], F32, tag="F", bufs=2, name=f"F{l}_{t}")
                nc.vector.tensor_add(F[:w], pf[:w, 0:257], wt[f"bva{l}"][:w])
                ew = sbuf.tile([P, 1], F32, tag="ew", bufs=2, name=f"ew{l}_{t}")
                nc.scalar.activation(ew[:w], F[:w, 256:257], AF.Lrelu,
                                     alpha=NEG_SLOPE)
                nc.scalar.activation(ew[:w], ew[:w], AF.Exp)
                stg = sbuf.tile([P, 257], BF16, tag="stg", bufs=2,
                                name=f"stg{l}_{t}")
                nc.vector.tensor_scalar_mul(out=stg[:w, 0:256],
                                            in0=F[:w, 0:256],
                                            scalar1=ew[:w, :])
                nc.vector.tensor_copy(out=stg[:w, 256:257], in_=ew[:w, :])
                nc.sync.dma_start(out=table[t * P:t * P + w, 0:257],
                                  in_=stg[:w, :])
                pi = psum.tile([P, 512], F32, tag="pd", bufs=1, name=f"pi{l}_{t}")
                for hi in range(nh):
                    nc.tensor.matmul(out=pi[:w, 0:256], lhsT=halves[hi],
                                     rhs=wt[f"Wx{l}h{hi}"][:],
                                     start=(hi == 0), stop=(hi == nh - 1))
                xi = sbuf.tile([P, 256], F32, tag="xi", bufs=2, name=f"xi{l}_{t}")
                nc.vector.tensor_add(xi[:w], pi[:w, 0:256], wt[f"bx{l}"][:w])
                nc.sync.dma_start(out=xinit_dram[t * P:t * P + w, :], in_=xi[:w])

        def edge_epilogue(l, rse, yin):
            for t in range(NT_MS):
                w = min(P, MS - t * P)
                rt = sbuf.tile([P, 257], BF16, tag="rt", bufs=2, name=f"rt{l}_{t}")
                nc.sync.dma_start(rt[:w], rse[t * P:t * P + w, :])
                dc = sbuf.tile([P, 1], F32, tag="dc", bufs=2, name=f"dc{l}_{t}")
                nc.vector.tensor_scalar_max(out=dc[:w], in0=rt[:w, 256:257],
                                            scalar1=1e-35)
                di = sbuf.tile([P, 1], F32, tag="di", bufs=2, name=f"di{l}_{t}")
                nc.vector.reciprocal(di[:w], dc[:w])
                z = sbuf.tile([P, 256], F32, tag="z", bufs=2, name=f"z{l}_{t}")
                nc.vector.tensor_scalar_mul(out=z[:w], in0=rt[:w, 0:256],
                                            scalar1=di[:w, :])
                u = elu_u(z[:w], w, 256, f"ee{l}", t)
                uT = []
                for hi in range(2):
                    pT = psum.tile([P, P], F32, tag="pT", bufs=1,
                                   name=f"pT{l}_{t}_{hi}")
                    nc.tensor.transpose(out=pT[:, 0:w],
                                        in_=u[:w, hi * P:(hi + 1) * P],
                                        identity=ident[:w, :w])
                    sT = sbuf.tile([P, P], F32, tag="sT", bufs=2,
                                   name=f"sT{l}_{t}_{hi}")
                    nc.vector.tensor_copy(out=sT[:, 0:w], in_=pT[:, 0:w])
                    uT.append(sT)
                py = psum.tile([P, 512], F32, tag="pd", bufs=1, name=f"py{l}_{t}")
                nc.tensor.matmul(out=py[:w, 0:256],
                                 lhsT=st_t[:, t * P:t * P + w],
                                 rhs=wt[f"Wt{l}bot"][:], start=True, stop=False)
                nc.tensor.matmul(out=py[:w, 0:256], lhsT=uT[0][:, 0:w],
                                 rhs=wt[f"Wt{l}h0"][:], start=False, stop=False)
                nc.tensor.matmul(out=py[:w, 0:256], lhsT=uT[1][:, 0:w],
                                 rhs=wt[f"Wt{l}h1"][:], start=False, stop=True)
                yt = sbuf.tile([P, 256], BF16, tag="yt", bufs=2, name=f"yt{l}_{t}")
                nc.vector.tensor_add(yt[:w], py[:w, 0:256], wt[f"bt{l}"][:w])
                nc.sync.dma_start(out=yin[t * P:t * P + w, :], in_=yt[:w])

        def node_epilogue(l, npart, xinit_dram, h_dram):
            for t in range(NT_V):
                w = min(P, NS - t * P)
                pt = sbuf.tile([P, 256], F32, tag="np", bufs=2, name=f"np{l}_{t}")
                nc.sync.dma_start(pt[:w], npart[t * P:t * P + w, 0:256])
                z = sbuf.tile([P, 256], F32, tag="nz", bufs=2, name=f"nz{l}_{t}")
                nc.vector.tensor_scalar_mul(out=z[:w], in0=pt[:w],
                                            scalar1=dgi_t[:w, t:t + 1])
                u = elu_u(z[:w], w, 256, f"ne{l}", t)
                xi = sbuf.tile([P, 256], F32, tag="nxi", bufs=2,
                               name=f"nxi{l}_{t}")
                nc.sync.dma_start(xi[:w], xinit_dram[t * P:t * P + w, :])
                h = sbuf.tile([P, 256], F32, tag="h", bufs=2, name=f"h{l}_{t}")
                nc.vector.tensor_add(h[:w], u[:w], xi[:w])
                nc.sync.dma_start(out=h_dram[t * P:t * P + w, :], in_=h[:w])
                for hi in range(2):
                    pT = psum.tile([P, P], F32, tag="pT", bufs=1,
                                   name=f"hpT{l}_{t}_{hi}")
                    nc.tensor.transpose(out=pT[:, 0:w],
                                        in_=h[:w, hi * P:(hi + 1) * P],
                                        identity=ident[:w, :w])
                    sT = sbuf.tile([P, P], F32, tag="sT", bufs=2,
                                   name=f"hsT{l}_{t}_{hi}")
                    nc.vector.tensor_copy(out=sT[:, 0:w], in_=pT[:, 0:w])
                    nc.sync.dma_start(out=hT[hi][:, t * P:t * P + w],
                                      in_=sT[:, 0:w])

        rg = [list(range(NCORES))]

        def dphgnn(l, lhsT_of, h_dram):
            table = dram.tile([NS, ESV], BF16, name=f"T{l}")
            xinit = dram.tile([NS, 256], F32, name=f"Xi{l}")
            dense_and_table(l, lhsT_of, table, xinit)
            part = dram.tile([NT_E * P, 257], BF16, name=f"part{l}")
            scatter_pass(table, ESV, 257, gvi_t, ecv_t, meta["cpt_v2e"], NT_E,
                         meta["regs_v"], part, 257, f"v{l}")
            rse = dram.tile([MS, 257], BF16, name=f"rse{l}")
            nc.gpsimd.collective_compute(
                "ReduceScatter", mybir.AluOpType.add, replica_groups=rg,
                ins=[part[0:M, :]], outs=[rse[:]])
            yin = dram.tile([MS, 256], BF16, name=f"yin{l}")
            edge_epilogue(l, rse, yin)
            ytab = dram.tile([M, ESE], BF16, name=f"ytab{l}")
            nc.gpsimd.collective_compute(
                "AllGather", mybir.AluOpType.bypass, replica_groups=rg,
                ins=[yin[:]], outs=[ytab[:]])
            npart = dram.tile([NT_V * P, 256], F32, name=f"npart{l}")
            scatter_pass(ytab, ESE, 256, gei_t, ece_t, meta["cpt_e2v"], NT_V,
                         meta["regs_e"], npart, 256, f"e{l}")
            node_epilogue(l, npart, xinit, h_dram)

        # layer 0
        h0 = dram.tile([NS, 256], F32, name="h0")

        def l0_of(t, w):
            xt = sbuf.tile([P, P], F32, tag="lhs", bufs=4, name=f"xt_{t}")
            nc.sync.dma_start(xt[:, 0:w], XT[:, t * P:t * P + w])
            return [xt[:, 0:w]]

        dphgnn(0, l0_of, h0)

        # layer 1
        h1 = dram.tile([NS, 256], F32, name="h1")

        def l1_of(t, w):
            out = []
            for hi in range(2):
                ht = sbuf.tile([P, P], F32, tag="lhs", bufs=4,
                               name=f"ht_{t}_{hi}")
                nc.sync.dma_start(ht[:, 0:w], hT[hi][:, t * P:t * P + w])
                out.append(ht[:, 0:w])
            return out

        dphgnn(1, l1_of, h1)

        # hyperconv
        table2 = dram.tile([NS, ESH], BF16, name="T2")
        for t in range(NT_V):
            w = min(P, NS - t * P)
            halves = l1_of(t, w)
            pf = psum.tile([P, 512], F32, tag="pd", bufs=1, name=f"pf2_{t}")
            for hi in range(2):
                nc.tensor.matmul(out=pf[:w, 0:128], lhsT=halves[hi],
                                 rhs=wt[f"Wfh{hi}"][:],
                                 start=(hi == 0), stop=(hi == 1))
            xf = sbuf.tile([P, 128], BF16, tag="xf", bufs=2, name=f"xf_{t}")
            nc.vector.tensor_add(xf[:w], pf[:w, 0:128], wt["bf"][:w])
            nc.sync.dma_start(out=table2[t * P:t * P + w, :], in_=xf[:w])
        part3 = dram.tile([NT_E * P, 128], BF16, name="part3")
        scatter_pass(table2, ESH, 128, gvi_t, ecv_t, meta["cpt_v2e"], NT_E,
                     meta["regs_v"], part3, 128, "v2")
        rse3 = dram.tile([MS, 128], BF16, name="rse3")
        nc.gpsimd.collective_compute(
            "ReduceScatter", mybir.AluOpType.add, replica_groups=rg,
            ins=[part3[0:M, :]], outs=[rse3[:]])
        yin3 = dram.tile([MS, 128], BF16, name="yin3")
        for t in range(NT_MS):
            w = min(P, MS - t * P)
            rt = sbuf.tile([P, 128], BF16, tag="rt3", bufs=2, name=f"rt3_{t}")
            nc.sync.dma_start(rt[:w], rse3[t * P:t * P + w, :])
            yt = sbuf.tile([P, 128], BF16, tag="yt3", bufs=2, name=f"yt3_{t}")
            nc.vector.tensor_scalar_mul(out=yt[:w], in0=rt[:w],
                                        scalar1=cf_t[:w, t:t + 1])
            nc.sync.dma_start(out=yin3[t * P:t * P + w, :], in_=yt[:w])
        ytab3 = dram.tile([M, 128], BF16, name="ytab3")
        nc.gpsimd.collective_compute(
            "AllGather", mybir.AluOpType.bypass, replica_groups=rg,
            ins=[yin3[:]], outs=[ytab3[:]])
        npart3 = dram.tile([NT_V * P, 128], F32, name="npart3")
        scatter_pass(ytab3, 128, 128, gei_t, ece_t, meta["cpt_e2v"], NT_V,
                     meta["regs_e"], npart3, 128, "e2")
        for t in range(NT_V):
            w = min(P, NS - t * P)
            pt = sbuf.tile([P, 128], F32, tag="fp", bufs=2, name=f"fp_{t}")
            nc.sync.dma_start(pt[:w], npart3[t * P:t * P + w, :])
            ot = sbuf.tile([P, 128], F32, tag="fo", bufs=2, name=f"fo_{t}")
            nc.vector.tensor_scalar_mul(out=ot[:w], in0=pt[:w],
                                        scalar1=dvi_t[:w, t:t + 1])
            nc.sync.dma_start(out=yout[t * P:t * P + w, :], in_=ot[:w])
        ctx.close()

    nc.compile()
    return nc


_CACHED = {}


def kernel(**inputs):
    in_maps, meta = _prep(inputs)
    key = (meta["LvP"], meta["LeP"], tuple(meta["cpt_v2e"]),
           tuple(meta["cpt_e2v"]))
    if key not in _CACHED:
        _CACHED[key] = build_program(meta)
    nc = _CACHED[key]
    res = run_bass_kernel_spmd(nc, in_maps, list(range(NCORES)))
    out = np.concatenate([res.results[c]["yout"] for c in range(NCORES)],
                         axis=0)
    return np.ascontiguousarray(out.astype(np.float32))


# revision 11
# speedup vs baseline: 1.0153x; 1.0153x over previous
"""Trainium2 Bass kernel for a 2-layer DPHGNN + hyperconv GNN message-passing stack.

kernel(**inputs) takes FULL numpy inputs (as produced by the problem's
setup_inputs()) and returns the FULL [50000, 128] float32 output.

Distribution: nodes (and their incident hypergraph entries) are sharded across
8 NeuronCores; edge-side tensors are exchanged with ReduceScatter / AllGather.
Gathers use the custom dma_gather SWDGE ucode; segment sums run on the tensor
engine via one-hot selection-matrix matmuls accumulated in PSUM.
"""

import sys
from contextlib import ExitStack

for _p in ("/opt/trn_rl_repo",):
    if _p not in sys.path:
        sys.path.append(_p)

import numpy as np

import concourse.bass as bass
import concourse.bacc as bacc
import concourse.mybir as mybir
import concourse.tile as tile
from concourse.bass_utils import run_bass_kernel_spmd
from concourse.masks import make_identity

F32 = mybir.dt.float32
BF16 = mybir.dt.bfloat16
I16 = mybir.dt.int16
AF = mybir.ActivationFunctionType

NEG_SLOPE = 0.2
P = 128
NCORES = 8
GQ = 4          # SWDGE queues
NI = 1024       # rows per dma_gather call (8 chunks of 128)
WCH = 8         # chunks per gather call / A-build batch
PSW = 1         # PSUM tiles per scatter mega-window


def _wrap_idx(flat):
    """[L] -> [128, L/16] int16: wrapped in 16 partitions, replicated x8."""
    L = len(flat)
    assert L % 16 == 0
    blk = np.asarray(flat, np.int16).reshape(-1, 16).T.copy()
    return np.ascontiguousarray(np.tile(blk, (8, 1)))


def _build_stream(dst, src_idx, n_tiles, cpt):
    """Destination-sorted, per-tile 128-padded entry stream.
    Returns (gidx[L], ec[L]) with ec = dst - tile*128 (pads: gidx 0, ec -1)."""
    order = np.argsort(dst, kind="stable")
    dsts = np.asarray(dst)[order]
    srcs = np.asarray(src_idx)[order]
    tile_of = dsts // P
    counts = np.bincount(tile_of, minlength=n_tiles)
    base = np.concatenate([[0], np.cumsum(cpt * P)])
    L = int(base[-1])
    gidx = np.zeros(L, np.int64)
    ec = -np.ones(L, np.float32)
    starts = np.concatenate([[0], np.cumsum(counts)])
    off = np.arange(len(dsts)) - starts[tile_of]
    slot = base[tile_of] + off
    gidx[slot] = srcs
    ec[slot] = dsts - tile_of * P
    return gidx, ec


def _prep(inputs):
    N, M = 50000, 20000
    NS, MS = N // NCORES, M // NCORES
    V = np.asarray(inputs["V"]).astype(np.int64)
    E = np.asarray(inputs["E"]).astype(np.int64)
    X = np.asarray(inputs["X"], np.float32)
    S = np.asarray(inputs["S"], np.float32)

    NT_E = (M + P - 1) // P           # 157
    NT_V = (NS + P - 1) // P          # 49
    NT_MS = (MS + P - 1) // P         # 20

    # graph-structure constants (mirroring the reference semantics)
    deg_v = np.bincount(V, minlength=N).astype(np.float64)
    cnt_e = np.bincount(E, minlength=M).astype(np.float64)
    deginv = np.where(deg_v > 0, 1.0 / np.maximum(deg_v, 1.0), 0.0)
    De = np.zeros(M, np.float64)
    np.add.at(De, E, deg_v[V])
    De = De / (cnt_e + 1.0)
    De_inv = np.where(De > 0, De ** -0.5, 1.0)
    coef_e = np.where(cnt_e > 0, De_inv / np.maximum(cnt_e, 1.0), De_inv * 0.0)
    Dv_inv = np.where(deg_v > 0, deg_v ** -0.5, 0.0)

    owner = V // NS
    v2e_raw, e2v_raw = [], []
    for c in range(NCORES):
        m = owner == c
        Vl = V[m] - c * NS
        Ee = E[m]
        v2e_raw.append((Ee, Vl))
        e2v_raw.append((Vl, Ee))

    def caps(raw, n_tiles):
        cpt = None
        for dst, _ in raw:
            counts = np.bincount(np.asarray(dst) // P, minlength=n_tiles)
            c1 = np.maximum(1, (counts + P - 1) // P)
            cpt = c1 if cpt is None else np.maximum(cpt, c1)
        return cpt

    cpt_v2e = caps(v2e_raw, NT_E)
    cpt_e2v = caps(e2v_raw, NT_V)
    Lv = int(np.sum(cpt_v2e) * P)
    Le = int(np.sum(cpt_e2v) * P)
    LvP = ((Lv + NI - 1) // NI) * NI
    LeP = ((Le + NI - 1) // NI) * NI

    def pad_stream(g, ec, LP):
        gi = np.full(LP, -1, np.int64)
        gi[: len(g)] = g
        ecp = np.full(LP, -1.0, np.float32)
        ecp[: len(ec)] = ec
        return gi, np.ascontiguousarray(ecp.reshape(-1, P).T)

    cores = []
    for c in range(NCORES):
        gv, ecv = _build_stream(*v2e_raw[c], NT_E, cpt_v2e)
        ge, ece = _build_stream(*e2v_raw[c], NT_V, cpt_e2v)
        gv_p, ecv_2d = pad_stream(gv, ecv, LvP)
        ge_p, ece_2d = pad_stream(ge, ece, LeP)
        cores.append(dict(gv_p=gv_p, ecv_2d=ecv_2d, ge_p=ge_p, ece_2d=ece_2d))

    def regs(L, LP):
        return [int(max(0, min(L - k * NI, NI))) for k in range(LP // NI)]

    regs_v, regs_e = regs(Lv, LvP), regs(Le, LeP)

    # folded weights
    g = lambda k: np.asarray(inputs[k], np.float32)
    W = {}
    for l in range(2):
        Wv, bv, a = g(f"Wv{l}"), g(f"bv{l}"), g(f"a{l}")
        Wx, bx = g(f"Wx{l}"), g(f"bx{l}")
        Wt, bt = g(f"Wt{l}"), g(f"bt{l}")
        Wva = np.concatenate([Wv, (Wv @ a)[:, None]], axis=1)
        bva = np.concatenate([bv, [float(bv @ a)]])
        Wt_top, Wt_bot = Wt[:256], Wt[256:]
        btf = bt - Wt_top.sum(axis=0)
        nh = Wva.shape[0] // P           # 1 (layer0) or 2 (layer1)
        for hi in range(nh):
            W[f"Wva{l}h{hi}"] = np.ascontiguousarray(Wva[hi * P:(hi + 1) * P])
            W[f"Wx{l}h{hi}"] = np.ascontiguousarray(Wx[hi * P:(hi + 1) * P])
        W[f"bva{l}"] = np.tile(bva[None, :].astype(np.float32), (P, 1))
        W[f"bx{l}"] = np.tile((bx - 1.0)[None, :], (P, 1))
        W[f"Wt{l}h0"] = np.ascontiguousarray(Wt_top[:128])
        W[f"Wt{l}h1"] = np.ascontiguousarray(Wt_top[128:])
        W[f"Wt{l}bot"] = np.ascontiguousarray(Wt_bot)
        W[f"bt{l}"] = np.tile(btf[None, :].astype(np.float32), (P, 1))
    Wf = g("Wf")
    W["Wfh0"] = np.ascontiguousarray(Wf[:128])
    W["Wfh1"] = np.ascontiguousarray(Wf[128:])
    W["bf"] = np.tile(g("bf")[None, :], (P, 1))

    iota = np.tile(np.arange(P, dtype=np.float32)[None, :], (P, 1))
    iota_rep = np.ascontiguousarray(
        np.broadcast_to(iota[:, None, :], (P, WCH, P)))

    def cols(arr_shard, n_tiles):
        out = np.zeros((P, n_tiles), np.float32)
        a = np.asarray(arr_shard, np.float32)
        for t in range(n_tiles):
            seg = a[t * P:(t + 1) * P]
            out[: len(seg), t] = seg
        return out

    in_maps = []
    for c in range(NCORES):
        d = cores[c]
        im = dict(
            XT=np.ascontiguousarray(X[c * NS:(c + 1) * NS].T),
            ST=np.ascontiguousarray(S[c * MS:(c + 1) * MS].T),
            gv_idx=_wrap_idx(d["gv_p"]), ge_idx=_wrap_idx(d["ge_p"]),
            ec_v=d["ecv_2d"], ec_e=d["ece_2d"],
            iota_rep=iota_rep,
            deginv_c=cols(deginv[c * NS:(c + 1) * NS], NT_V),
            dvinv_c=cols(Dv_inv[c * NS:(c + 1) * NS], NT_V),
            coef_c=cols(coef_e[c * MS:(c + 1) * MS], NT_MS),
        )
        im.update(W)
        in_maps.append(im)

    meta = dict(N=N, M=M, NS=NS, MS=MS, NT_E=NT_E, NT_V=NT_V, NT_MS=NT_MS,
                cpt_v2e=[int(x) for x in cpt_v2e],
                cpt_e2v=[int(x) for x in cpt_e2v],
                LvP=LvP, LeP=LeP, regs_v=regs_v, regs_e=regs_e)
    return in_maps, meta


# ---------------------------------------------------------------------------

def build_program(meta):
    NS, MS, M = meta["NS"], meta["MS"], meta["M"]
    NT_E, NT_V, NT_MS = meta["NT_E"], meta["NT_V"], meta["NT_MS"]
    ESV, ESE, ESH = 384, 256, 128

    nc = bacc.Bacc("TRN2", target_bir_lowering=False, debug=False,
                   num_devices=NCORES, num_swdge_queues=GQ)

    def din(name, shape, dt=F32):
        return nc.dram_tensor(name, shape, dt, kind="ExternalInput")

    XT = din("XT", [P, NS])
    ST = din("ST", [64, MS])
    gv_idx = din("gv_idx", [P, meta["LvP"] // 16], I16)
    ge_idx = din("ge_idx", [P, meta["LeP"] // 16], I16)
    nch_v = meta["LvP"] // P
    nch_e = meta["LeP"] // P
    ec_v = din("ec_v", [P, nch_v])
    ec_e = din("ec_e", [P, nch_e])
    iota_rep = din("iota_rep", [P, WCH, P])
    deginv_c = din("deginv_c", [P, NT_V])
    dvinv_c = din("dvinv_c", [P, NT_V])
    coef_c = din("coef_c", [P, NT_MS])
    wnames = (["Wva0h0", "Wx0h0", "Wva1h0", "Wva1h1", "Wx1h0", "Wx1h1",
               "Wt0h0", "Wt0h1", "Wt1h0", "Wt1h1", "Wfh0", "Wfh1"],
              ["bva0", "bx0", "bva1", "bx1", "bt0", "bt1", "bf"],
              ["Wt0bot", "Wt1bot"])
    wshapes = dict(Wva0h0=[P, 257], Wx0h0=[P, 256],
                   Wva1h0=[P, 257], Wva1h1=[P, 257],
                   Wx1h0=[P, 256], Wx1h1=[P, 256],
                   Wt0h0=[P, 256], Wt0h1=[P, 256],
                   Wt1h0=[P, 256], Wt1h1=[P, 256],
                   Wfh0=[P, 128], Wfh1=[P, 128],
                   bva0=[P, 257], bx0=[P, 256], bva1=[P, 257], bx1=[P, 256],
                   bt0=[P, 256], bt1=[P, 256], bf=[P, 128],
                   Wt0bot=[64, 256], Wt1bot=[64, 256])
    Wd = {k: din(k, wshapes[k]) for grp in wnames for k in grp}

    yout = nc.dram_tensor("yout", [NS, 128], F32, kind="ExternalOutput")

    with tile.TileContext(nc) as tc:
        ctx = ExitStack()
        sbuf = ctx.enter_context(tc.tile_pool(name="sbuf", bufs=2))
        psum = ctx.enter_context(tc.tile_pool(name="psum", bufs=2, space="PSUM"))
        dram = ctx.enter_context(tc.tile_pool(name="dram", bufs=1, space="DRAM"))
        cons = ctx.enter_context(tc.tile_pool(name="cons", bufs=1))

        iota_t = cons.tile([P, WCH, P], F32, name="iota_t")
        nc.sync.dma_start(iota_t[:], iota_rep[:])
        ident = cons.tile([P, P], F32, name="ident")
        make_identity(nc, ident[:])
        wt = {}
        for k, h in Wd.items():
            t = cons.tile(list(h.shape), F32, name=f"w_{k}")
            nc.sync.dma_start(t[:], h[:])
            wt[k] = t
        st_t = cons.tile([64, MS], F32, name="st_t")
        nc.sync.dma_start(st_t[:], ST[:])
        ecv_t = cons.tile([P, nch_v], F32, name="ecv_t")
        nc.sync.dma_start(ecv_t[:], ec_v[:])
        ece_t = cons.tile([P, nch_e], F32, name="ece_t")
        nc.sync.dma_start(ece_t[:], ec_e[:])
        gvi_t = cons.tile([P, meta["LvP"] // 16], I16, name="gvi_t")
        nc.sync.dma_start(gvi_t[:], gv_idx[:])
        gei_t = cons.tile([P, meta["LeP"] // 16], I16, name="gei_t")
        nc.sync.dma_start(gei_t[:], ge_idx[:])
        dgi_t = cons.tile([P, NT_V], F32, name="dgi_t")
        nc.sync.dma_start(dgi_t[:], deginv_c[:])
        dvi_t = cons.tile([P, NT_V], F32, name="dvi_t")
        nc.sync.dma_start(dvi_t[:], dvinv_c[:])
        cf_t = cons.tile([P, NT_MS], F32, name="cf_t")
        nc.sync.dma_start(cf_t[:], coef_c[:])

        hT = [dram.tile([P, NS], F32, name="hT0"),
              dram.tile([P, NS], F32, name="hT1")]

        def scatter_pass(table, es, used_cols, idx_t, ec_t, cpt, n_tiles,
                         regs, out_dram, out_cols, tag):
            tile_of_chunk = []
            for t, n in enumerate(cpt):
                tile_of_chunk += [t] * n
            n_chunks = len(tile_of_chunk)
            first_c, last_c = {}, {}
            for k, t in enumerate(tile_of_chunk):
                first_c.setdefault(t, k)
                last_c[t] = k
            g_tiles = []
            for call in range(len(regs)):
                if regs[call] == 0:
                    g_tiles.append(None)
                    continue
                gt = sbuf.tile([P, WCH, es], BF16, tag="g", bufs=8,
                               name=f"g{tag}_{call}")
                nc.gpsimd.dma_gather(
                    out_ap=gt[:], in_ap=table[:],
                    idxs_ap=idx_t[:, call * (NI // 16):(call + 1) * (NI // 16)],
                    num_idxs=NI, num_idxs_reg=regs[call], elem_size=es,
                    queue_num=call % GQ)
                g_tiles.append(gt)
            a_cur = [None, -1]
            mega = [None, -1]   # tile, window index
            stg_cur = [None]
            for k in range(n_chunks):
                t = tile_of_chunk[k]
                call, j = k // WCH, k % WCH
                gt = g_tiles[call]
                if gt is None:
                    continue
                w = k // WCH
                if a_cur[1] != w:
                    ab = sbuf.tile([P, WCH, P], BF16, tag="A", bufs=6,
                                   name=f"A{tag}_{w}")
                    nc.vector.tensor_tensor(
                        out=ab[:],
                        in0=ec_t[:, w * WCH:(w + 1) * WCH].to_broadcast(
                            [P, WCH, P]),
                        in1=iota_t[:],
                        op=mybir.AluOpType.is_equal)
                    a_cur = [ab, w]
                mw = t // PSW
                if mega[1] != mw:
                    mega = [psum.tile([P, PSW, 512], F32, tag="ps", bufs=4,
                                      name=f"ps{tag}_{mw}"), mw]
                pt = mega[0]
                q = t % PSW
                nc.tensor.matmul(
                    out=pt[:, q, 0:out_cols],
                    lhsT=a_cur[0][:, j, :],
                    rhs=gt[:, j, 0:used_cols],
                    start=(k == first_c[t]), stop=(k == last_c[t]))
                if k == last_c[t]:
                    if stg_cur[0] is None:
                        stg_cur[0] = sbuf.tile([P, PSW, out_cols],
                                               out_dram.dtype,
                                               tag="pstg", bufs=4,
                                               name=f"pstg{tag}_{t}")
                    nc.vector.tensor_copy(out=stg_cur[0][:, q, :],
                                          in_=pt[:, q, 0:out_cols])
                    if t % PSW == PSW - 1 or t == n_tiles - 1:
                        n_in_w = q + 1
                        rows0 = (t - n_in_w + 1) * P
                        nc.sync.dma_start(
                            out=out_dram[rows0:rows0 + n_in_w * P, :].rearrange(
                                "(j p) c -> p j c", p=P),
                            in_=stg_cur[0][:, 0:n_in_w, :])
                        stg_cur[0] = None

        def elu_u(z_ap, w, cols, tag, i):
            mn = sbuf.tile([P, cols], F32, tag="mn", bufs=2, name=f"mn{tag}{i}")
            nc.vector.tensor_scalar_min(out=mn[:w], in0=z_ap, scalar1=0.0)
            ex = sbuf.tile([P, cols], F32, tag="ex", bufs=2, name=f"ex{tag}{i}")
            nc.scalar.activation(ex[:w], mn[:w], AF.Exp)
            rl = sbuf.tile([P, cols], F32, tag="rl", bufs=2, name=f"rl{tag}{i}")
            nc.scalar.activation(rl[:w], z_ap, AF.Relu)
            u = sbuf.tile([P, cols], F32, tag="u", bufs=2, name=f"u{tag}{i}")
            nc.vector.tensor_add(u[:w], rl[:w], ex[:w])
            return u

        def dense_and_table(l, lhsT_of, table, xinit_dram):
            nh = 1 if l == 0 else 2
            for t in range(NT_V):
                w = min(P, NS - t * P)
                halves = lhsT_of(t, w)
                pf = psum.tile([P, 512], F32, tag="pd", bufs=2, name=f"pf{l}_{t}")
                for hi in range(nh):
                    nc.tensor.matmul(out=pf[:w, 0:257], lhsT=halves[hi],
                                     rhs=wt[f"Wva{l}h{hi}"][:],
                                     start=(hi == 0), stop=(hi == nh - 1))
                F = sbuf.tile([P, 257], F32, tag="F", bufs=2, name=f"F{l}_{t}")
                nc.vector.tensor_add(F[:w], pf[:w, 0:257], wt[f"bva{l}"][:w])
                ew = sbuf.tile([P, 1], F32, tag="ew", bufs=2, name=f"ew{l}_{t}")
                nc.scalar.activation(ew[:w], F[:w, 256:257], AF.Lrelu,
                                     alpha=NEG_SLOPE)
                nc.scalar.activation(ew[:w], ew[:w], AF.Exp)
                stg = sbuf.tile([P, 257], BF16, tag="stg", bufs=2,
                                name=f"stg{l}_{t}")
                nc.vector.tensor_scalar_mul(out=stg[:w, 0:256],
                                            in0=F[:w, 0:256],
                                            scalar1=ew[:w, :])
                nc.vector.tensor_copy(out=stg[:w, 256:257], in_=ew[:w, :])
                nc.sync.dma_start(out=table[t * P:t * P + w, 0:257],
                                  in_=stg[:w, :])
                pi = psum.tile([P, 512], F32, tag="pd", bufs=2, name=f"pi{l}_{t}")
                for hi in range(nh):
                    nc.tensor.matmul(out=pi[:w, 0:256], lhsT=halves[hi],
                                     rhs=wt[f"Wx{l}h{hi}"][:],
                                     start=(hi == 0), stop=(hi == nh - 1))
                xi = sbuf.tile([P, 256], F32, tag="xi", bufs=2, name=f"xi{l}_{t}")
                nc.vector.tensor_add(xi[:w], pi[:w, 0:256], wt[f"bx{l}"][:w])
                nc.sync.dma_start(out=xinit_dram[t * P:t * P + w, :], in_=xi[:w])

        def edge_epilogue(l, rse, yin):
            for t in range(NT_MS):
                w = min(P, MS - t * P)
                rt = sbuf.tile([P, 257], BF16, tag="rt", bufs=2, name=f"rt{l}_{t}")
                nc.sync.dma_start(rt[:w], rse[t * P:t * P + w, :])
                dc = sbuf.tile([P, 1], F32, tag="dc", bufs=2, name=f"dc{l}_{t}")
                nc.vector.tensor_scalar_max(out=dc[:w], in0=rt[:w, 256:257],
                                            scalar1=1e-35)
                di = sbuf.tile([P, 1], F32, tag="di", bufs=2, name=f"di{l}_{t}")
                nc.vector.reciprocal(di[:w], dc[:w])
                z = sbuf.tile([P, 256], F32, tag="z", bufs=2, name=f"z{l}_{t}")
                nc.vector.tensor_scalar_mul(out=z[:w], in0=rt[:w, 0:256],
                                            scalar1=di[:w, :])
                u = elu_u(z[:w], w, 256, f"ee{l}", t)
                uT = []
                for hi in range(2):
                    pT = psum.tile([P, P], F32, tag="pT", bufs=2,
                                   name=f"pT{l}_{t}_{hi}")
                    nc.tensor.transpose(out=pT[:, 0:w],
                                        in_=u[:w, hi * P:(hi + 1) * P],
                                        identity=ident[:w, :w])
                    sT = sbuf.tile([P, P], F32, tag="sT", bufs=2,
                                   name=f"sT{l}_{t}_{hi}")
                    nc.vector.tensor_copy(out=sT[:, 0:w], in_=pT[:, 0:w])
                    uT.append(sT)
                py = psum.tile([P, 512], F32, tag="pd", bufs=2, name=f"py{l}_{t}")
                nc.tensor.matmul(out=py[:w, 0:256],
                                 lhsT=st_t[:, t * P:t * P + w],
                                 rhs=wt[f"Wt{l}bot"][:], start=True, stop=False)
                nc.tensor.matmul(out=py[:w, 0:256], lhsT=uT[0][:, 0:w],
                                 rhs=wt[f"Wt{l}h0"][:], start=False, stop=False)
                nc.tensor.matmul(out=py[:w, 0:256], lhsT=uT[1][:, 0:w],
                                 rhs=wt[f"Wt{l}h1"][:], start=False, stop=True)
                yt = sbuf.tile([P, 256], BF16, tag="yt", bufs=2, name=f"yt{l}_{t}")
                nc.vector.tensor_add(yt[:w], py[:w, 0:256], wt[f"bt{l}"][:w])
                nc.sync.dma_start(out=yin[t * P:t * P + w, :], in_=yt[:w])

        def node_epilogue(l, npart, xinit_dram, h_dram):
            for t in range(NT_V):
                w = min(P, NS - t * P)
                pt = sbuf.tile([P, 256], F32, tag="np", bufs=2, name=f"np{l}_{t}")
                nc.sync.dma_start(pt[:w], npart[t * P:t * P + w, 0:256])
                z = sbuf.tile([P, 256], F32, tag="nz", bufs=2, name=f"nz{l}_{t}")
                nc.vector.tensor_scalar_mul(out=z[:w], in0=pt[:w],
                                            scalar1=dgi_t[:w, t:t + 1])
                u = elu_u(z[:w], w, 256, f"ne{l}", t)
                xi = sbuf.tile([P, 256], F32, tag="nxi", bufs=2,
                               name=f"nxi{l}_{t}")
                nc.sync.dma_start(xi[:w], xinit_dram[t * P:t * P + w, :])
                h = sbuf.tile([P, 256], F32, tag="h", bufs=2, name=f"h{l}_{t}")
                nc.vector.tensor_add(h[:w], u[:w], xi[:w])
                nc.sync.dma_start(out=h_dram[t * P:t * P + w, :], in_=h[:w])
                for hi in range(2):
                    pT = psum.tile([P, P], F32, tag="pT", bufs=2,
                                   name=f"hpT{l}_{t}_{hi}")
                    nc.tensor.transpose(out=pT[:, 0:w],
                                        in_=h[:w, hi * P:(hi + 1) * P],
                                        identity=ident[:w, :w])
                    sT = sbuf.tile([P, P], F32, tag="sT", bufs=2,
                                   name=f"hsT{l}_{t}_{hi}")
                    nc.vector.tensor_copy(out=sT[:, 0:w], in_=pT[:, 0:w])
                    nc.sync.dma_start(out=hT[hi][:, t * P:t * P + w],
                                      in_=sT[:, 0:w])

        rg = [list(range(NCORES))]

        def dphgnn(l, lhsT_of, h_dram):
            table = dram.tile([NS, ESV], BF16, name=f"T{l}")
            xinit = dram.tile([NS, 256], F32, name=f"Xi{l}")
            dense_and_table(l, lhsT_of, table, xinit)
            part = dram.tile([NT_E * P, 257], BF16, name=f"part{l}")
            scatter_pass(table, ESV, 257, gvi_t, ecv_t, meta["cpt_v2e"], NT_E,
                         meta["regs_v"], part, 257, f"v{l}")
            rse = dram.tile([MS, 257], BF16, name=f"rse{l}")
            nc.gpsimd.collective_compute(
                "ReduceScatter", mybir.AluOpType.add, replica_groups=rg,
                ins=[part[0:M, :]], outs=[rse[:]])
            yin = dram.tile([MS, 256], BF16, name=f"yin{l}")
            edge_epilogue(l, rse, yin)
            ytab = dram.tile([M, ESE], BF16, name=f"ytab{l}")
            nc.gpsimd.collective_compute(
                "AllGather", mybir.AluOpType.bypass, replica_groups=rg,
                ins=[yin[:]], outs=[ytab[:]])
            npart = dram.tile([NT_V * P, 256], F32, name=f"npart{l}")
            scatter_pass(ytab, ESE, 256, gei_t, ece_t, meta["cpt_e2v"], NT_V,
                         meta["regs_e"], npart, 256, f"e{l}")
            node_epilogue(l, npart, xinit, h_dram)

        # layer 0
        h0 = dram.tile([NS, 256], F32, name="h0")

        def l0_of(t, w):
            xt = sbuf.tile([P, P], F32, tag="lhs", bufs=4, name=f"xt_{t}")
            nc.sync.dma_start(xt[:, 0:w], XT[:, t * P:t * P + w])
            return [xt[:, 0:w]]

        dphgnn(0, l0_of, h0)

        # layer 1
        h1 = dram.tile([NS, 256], F32, name="h1")

        def l1_of(t, w):
            out = []
            for hi in range(2):
                ht = sbuf.tile([P, P], F32, tag="lhs", bufs=4,
                               name=f"ht_{t}_{hi}")
                nc.sync.dma_start(ht[:, 0:w], hT[hi][:, t * P:t * P + w])
                out.append(ht[:, 0:w])
            return out

        dphgnn(1, l1_of, h1)

        # hyperconv
        table2 = dram.tile([NS, ESH], BF16, name="T2")
        for t in range(NT_V):
            w = min(P, NS - t * P)
            halves = l1_of(t, w)
            pf = psum.tile([P, 512], F32, tag="pd", bufs=2, name=f"pf2_{t}")
            for hi in range(2):
                nc.tensor.matmul(out=pf[:w, 0:128], lhsT=halves[hi],
                                 rhs=wt[f"Wfh{hi}"][:],
                                 start=(hi == 0), stop=(hi == 1))
            xf = sbuf.tile([P, 128], BF16, tag="xf", bufs=2, name=f"xf_{t}")
            nc.vector.tensor_add(xf[:w], pf[:w, 0:128], wt["bf"][:w])
            nc.sync.dma_start(out=table2[t * P:t * P + w, :], in_=xf[:w])
        part3 = dram.tile([NT_E * P, 128], BF16, name="part3")
        scatter_pass(table2, ESH, 128, gvi_t, ecv_t, meta["cpt_v2e"], NT_E,
                     meta["regs_v"], part3, 128, "v2")
        rse3 = dram.tile([MS, 128], BF16, name="rse3")
        nc.gpsimd.collective_compute(
            "ReduceScatter", mybir.AluOpType.add, replica_groups=rg,
            ins=[part3[0:M, :]], outs=[rse3[:]])
        yin3 = dram.tile([MS, 128], BF16, name="yin3")
        for t in range(NT_MS):
            w = min(P, MS - t * P)
            rt = sbuf.tile([P, 128], BF16, tag="rt3", bufs=2, name=f"rt3_{t}")
            nc.sync.dma_start(rt[:w], rse3[t * P:t * P + w, :])
            yt = sbuf.tile([P, 128], BF16, tag="yt3", bufs=2, name=f"yt3_{t}")
            nc.vector.tensor_scalar_mul(out=yt[:w], in0=rt[:w],
                                        scalar1=cf_t[:w, t:t + 1])
            nc.sync.dma_start(out=yin3[t * P:t * P + w, :], in_=yt[:w])
        ytab3 = dram.tile([M, 128], BF16, name="ytab3")
        nc.gpsimd.collective_compute(
            "AllGather", mybir.AluOpType.bypass, replica_groups=rg,
            ins=[yin3[:]], outs=[ytab3[:]])
        npart3 = dram.tile([NT_V * P, 128], F32, name="npart3")
        scatter_pass(ytab3, 128, 128, gei_t, ece_t, meta["cpt_e2v"], NT_V,
                     meta["regs_e"], npart3, 128, "e2")
        for t in range(NT_V):
            w = min(P, NS - t * P)
            pt = sbuf.tile([P, 128], F32, tag="fp", bufs=2, name=f"fp_{t}")
            nc.sync.dma_start(pt[:w], npart3[t * P:t * P + w, :])
            ot = sbuf.tile([P, 128], F32, tag="fo", bufs=2, name=f"fo_{t}")
            nc.vector.tensor_scalar_mul(out=ot[:w], in0=pt[:w],
                                        scalar1=dvi_t[:w, t:t + 1])
            nc.sync.dma_start(out=yout[t * P:t * P + w, :], in_=ot[:w])
        ctx.close()

    nc.compile()
    return nc


_CACHED = {}


def kernel(**inputs):
    in_maps, meta = _prep(inputs)
    key = (meta["LvP"], meta["LeP"], tuple(meta["cpt_v2e"]),
           tuple(meta["cpt_e2v"]))
    if key not in _CACHED:
        _CACHED[key] = build_program(meta)
    nc = _CACHED[key]
    res = run_bass_kernel_spmd(nc, in_maps, list(range(NCORES)))
    out = np.concatenate([res.results[c]["yout"] for c in range(NCORES)],
                         axis=0)
    return np.ascontiguousarray(out.astype(np.float32))


# revision 13
# speedup vs baseline: 1.0207x; 1.0054x over previous
"""Trainium2 Bass kernel for a 2-layer DPHGNN + hyperconv GNN message-passing stack.

kernel(**inputs) takes FULL numpy inputs (as produced by the problem's
setup_inputs()) and returns the FULL [50000, 128] float32 output.

Distribution: nodes (and their incident hypergraph entries) are sharded across
8 NeuronCores; edge-side tensors are exchanged with ReduceScatter / AllGather.
Gathers use the custom dma_gather SWDGE ucode; segment sums run on the tensor
engine via one-hot selection-matrix matmuls accumulated in PSUM.
"""

import sys
from contextlib import ExitStack

for _p in ("/opt/trn_rl_repo",):
    if _p not in sys.path:
        sys.path.append(_p)

import numpy as np

import concourse.bass as bass
import concourse.bacc as bacc
import concourse.mybir as mybir
import concourse.tile as tile
from concourse.bass_utils import run_bass_kernel_spmd
from concourse.masks import make_identity

F32 = mybir.dt.float32
BF16 = mybir.dt.bfloat16
I16 = mybir.dt.int16
AF = mybir.ActivationFunctionType

NEG_SLOPE = 0.2
P = 128
NCORES = 8
GQ = 4          # SWDGE queues
NI = 1024       # rows per dma_gather call (8 chunks of 128)
WCH = 8         # chunks per gather call / A-build batch
PSW = 2         # PSUM tiles per scatter mega-window


def _wrap_idx(flat):
    """[L] -> [128, L/16] int16: wrapped in 16 partitions, replicated x8."""
    L = len(flat)
    assert L % 16 == 0
    blk = np.asarray(flat, np.int16).reshape(-1, 16).T.copy()
    return np.ascontiguousarray(np.tile(blk, (8, 1)))


def _build_stream(dst, src_idx, n_tiles, cpt):
    """Destination-sorted, per-tile 128-padded entry stream.
    Returns (gidx[L], ec[L]) with ec = dst - tile*128 (pads: gidx 0, ec -1)."""
    order = np.argsort(dst, kind="stable")
    dsts = np.asarray(dst)[order]
    srcs = np.asarray(src_idx)[order]
    tile_of = dsts // P
    counts = np.bincount(tile_of, minlength=n_tiles)
    base = np.concatenate([[0], np.cumsum(cpt * P)])
    L = int(base[-1])
    gidx = np.zeros(L, np.int64)
    ec = -np.ones(L, np.float32)
    starts = np.concatenate([[0], np.cumsum(counts)])
    off = np.arange(len(dsts)) - starts[tile_of]
    slot = base[tile_of] + off
    gidx[slot] = srcs
    ec[slot] = dsts - tile_of * P
    return gidx, ec


def _prep(inputs):
    N, M = 50000, 20000
    NS, MS = N // NCORES, M // NCORES
    V = np.asarray(inputs["V"]).astype(np.int64)
    E = np.asarray(inputs["E"]).astype(np.int64)
    X = np.asarray(inputs["X"], np.float32)
    S = np.asarray(inputs["S"], np.float32)

    NT_E = (M + P - 1) // P           # 157
    NT_V = (NS + P - 1) // P          # 49
    NT_MS = (MS + P - 1) // P         # 20

    # graph-structure constants (mirroring the reference semantics)
    deg_v = np.bincount(V, minlength=N).astype(np.float64)
    cnt_e = np.bincount(E, minlength=M).astype(np.float64)
    deginv = np.where(deg_v > 0, 1.0 / np.maximum(deg_v, 1.0), 0.0)
    De = np.zeros(M, np.float64)
    np.add.at(De, E, deg_v[V])
    De = De / (cnt_e + 1.0)
    De_inv = np.where(De > 0, De ** -0.5, 1.0)
    coef_e = np.where(cnt_e > 0, De_inv / np.maximum(cnt_e, 1.0), De_inv * 0.0)
    Dv_inv = np.where(deg_v > 0, deg_v ** -0.5, 0.0)

    owner = V // NS
    v2e_raw, e2v_raw = [], []
    for c in range(NCORES):
        m = owner == c
        Vl = V[m] - c * NS
        Ee = E[m]
        v2e_raw.append((Ee, Vl))
        e2v_raw.append((Vl, Ee))

    def caps(raw, n_tiles):
        cpt = None
        for dst, _ in raw:
            counts = np.bincount(np.asarray(dst) // P, minlength=n_tiles)
            c1 = np.maximum(1, (counts + P - 1) // P)
            cpt = c1 if cpt is None else np.maximum(cpt, c1)
        return cpt

    cpt_v2e = caps(v2e_raw, NT_E)
    cpt_e2v = caps(e2v_raw, NT_V)
    Lv = int(np.sum(cpt_v2e) * P)
    Le = int(np.sum(cpt_e2v) * P)
    LvP = ((Lv + NI - 1) // NI) * NI
    LeP = ((Le + NI - 1) // NI) * NI

    def pad_stream(g, ec, LP):
        gi = np.full(LP, -1, np.int64)
        gi[: len(g)] = g
        ecp = np.full(LP, -1.0, np.float32)
        ecp[: len(ec)] = ec
        return gi, np.ascontiguousarray(ecp.reshape(-1, P).T)

    cores = []
    for c in range(NCORES):
        gv, ecv = _build_stream(*v2e_raw[c], NT_E, cpt_v2e)
        ge, ece = _build_stream(*e2v_raw[c], NT_V, cpt_e2v)
        gv_p, ecv_2d = pad_stream(gv, ecv, LvP)
        ge_p, ece_2d = pad_stream(ge, ece, LeP)
        cores.append(dict(gv_p=gv_p, ecv_2d=ecv_2d, ge_p=ge_p, ece_2d=ece_2d))

    def regs(L, LP):
        return [int(max(0, min(L - k * NI, NI))) for k in range(LP // NI)]

    regs_v, regs_e = regs(Lv, LvP), regs(Le, LeP)

    # folded weights
    g = lambda k: np.asarray(inputs[k], np.float32)
    W = {}
    for l in range(2):
        Wv, bv, a = g(f"Wv{l}"), g(f"bv{l}"), g(f"a{l}")
        Wx, bx = g(f"Wx{l}"), g(f"bx{l}")
        Wt, bt = g(f"Wt{l}"), g(f"bt{l}")
        Wva = np.concatenate([Wv, (Wv @ a)[:, None]], axis=1)
        bva = np.concatenate([bv, [float(bv @ a)]])
        Wt_top, Wt_bot = Wt[:256], Wt[256:]
        btf = bt - Wt_top.sum(axis=0)
        nh = Wva.shape[0] // P           # 1 (layer0) or 2 (layer1)
        for hi in range(nh):
            W[f"Wva{l}h{hi}"] = np.ascontiguousarray(Wva[hi * P:(hi + 1) * P])
            W[f"Wx{l}h{hi}"] = np.ascontiguousarray(Wx[hi * P:(hi + 1) * P])
        W[f"bva{l}"] = np.tile(bva[None, :].astype(np.float32), (P, 1))
        W[f"bx{l}"] = np.tile((bx - 1.0)[None, :], (P, 1))
        W[f"Wt{l}h0"] = np.ascontiguousarray(Wt_top[:128])
        W[f"Wt{l}h1"] = np.ascontiguousarray(Wt_top[128:])
        W[f"Wt{l}bot"] = np.ascontiguousarray(Wt_bot)
        W[f"bt{l}"] = np.tile(btf[None, :].astype(np.float32), (P, 1))
    Wf = g("Wf")
    W["Wfh0"] = np.ascontiguousarray(Wf[:128])
    W["Wfh1"] = np.ascontiguousarray(Wf[128:])
    W["bf"] = np.tile(g("bf")[None, :], (P, 1))

    iota = np.tile(np.arange(P, dtype=np.float32)[None, :], (P, 1))
    iota_rep = np.ascontiguousarray(
        np.broadcast_to(iota[:, None, :], (P, WCH, P)))

    def cols(arr_shard, n_tiles):
        out = np.zeros((P, n_tiles), np.float32)
        a = np.asarray(arr_shard, np.float32)
        for t in range(n_tiles):
            seg = a[t * P:(t + 1) * P]
            out[: len(seg), t] = seg
        return out

    in_maps = []
    for c in range(NCORES):
        d = cores[c]
        im = dict(
            XT=np.ascontiguousarray(X[c * NS:(c + 1) * NS].T),
            ST=np.ascontiguousarray(S[c * MS:(c + 1) * MS].T),
            gv_idx=_wrap_idx(d["gv_p"]), ge_idx=_wrap_idx(d["ge_p"]),
            ec_v=d["ecv_2d"], ec_e=d["ece_2d"],
            iota_rep=iota_rep,
            deginv_c=cols(deginv[c * NS:(c + 1) * NS], NT_V),
            dvinv_c=cols(Dv_inv[c * NS:(c + 1) * NS], NT_V),
            coef_c=cols(coef_e[c * MS:(c + 1) * MS], NT_MS),
        )
        im.update(W)
        in_maps.append(im)

    meta = dict(N=N, M=M, NS=NS, MS=MS, NT_E=NT_E, NT_V=NT_V, NT_MS=NT_MS,
                cpt_v2e=[int(x) for x in cpt_v2e],
                cpt_e2v=[int(x) for x in cpt_e2v],
                LvP=LvP, LeP=LeP, regs_v=regs_v, regs_e=regs_e)
    return in_maps, meta


# ---------------------------------------------------------------------------

def build_program(meta):
    NS, MS, M = meta["NS"], meta["MS"], meta["M"]
    NT_E, NT_V, NT_MS = meta["NT_E"], meta["NT_V"], meta["NT_MS"]
    ESV, ESE, ESH = 384, 256, 128

    nc = bacc.Bacc("TRN2", target_bir_lowering=False, debug=False,
                   num_devices=NCORES, num_swdge_queues=GQ)

    def din(name, shape, dt=F32):
        return nc.dram_tensor(name, shape, dt, kind="ExternalInput")

    XT = din("XT", [P, NS])
    ST = din("ST", [64, MS])
    gv_idx = din("gv_idx", [P, meta["LvP"] // 16], I16)
    ge_idx = din("ge_idx", [P, meta["LeP"] // 16], I16)
    nch_v = meta["LvP"] // P
    nch_e = meta["LeP"] // P
    ec_v = din("ec_v", [P, nch_v])
    ec_e = din("ec_e", [P, nch_e])
    iota_rep = din("iota_rep", [P, WCH, P])
    deginv_c = din("deginv_c", [P, NT_V])
    dvinv_c = din("dvinv_c", [P, NT_V])
    coef_c = din("coef_c", [P, NT_MS])
    wnames = (["Wva0h0", "Wx0h0", "Wva1h0", "Wva1h1", "Wx1h0", "Wx1h1",
               "Wt0h0", "Wt0h1", "Wt1h0", "Wt1h1", "Wfh0", "Wfh1"],
              ["bva0", "bx0", "bva1", "bx1", "bt0", "bt1", "bf"],
              ["Wt0bot", "Wt1bot"])
    wshapes = dict(Wva0h0=[P, 257], Wx0h0=[P, 256],
                   Wva1h0=[P, 257], Wva1h1=[P, 257],
                   Wx1h0=[P, 256], Wx1h1=[P, 256],
                   Wt0h0=[P, 256], Wt0h1=[P, 256],
                   Wt1h0=[P, 256], Wt1h1=[P, 256],
                   Wfh0=[P, 128], Wfh1=[P, 128],
                   bva0=[P, 257], bx0=[P, 256], bva1=[P, 257], bx1=[P, 256],
                   bt0=[P, 256], bt1=[P, 256], bf=[P, 128],
                   Wt0bot=[64, 256], Wt1bot=[64, 256])
    Wd = {k: din(k, wshapes[k]) for grp in wnames for k in grp}

    yout = nc.dram_tensor("yout", [NS, 128], F32, kind="ExternalOutput")

    with tile.TileContext(nc) as tc:
        ctx = ExitStack()
        sbuf = ctx.enter_context(tc.tile_pool(name="sbuf", bufs=2))
        psum = ctx.enter_context(tc.tile_pool(name="psum", bufs=2, space="PSUM"))
        dram = ctx.enter_context(tc.tile_pool(name="dram", bufs=1, space="DRAM"))
        cons = ctx.enter_context(tc.tile_pool(name="cons", bufs=1))

        iota_t = cons.tile([P, WCH, P], F32, name="iota_t")
        nc.sync.dma_start(iota_t[:], iota_rep[:])
        ident = cons.tile([P, P], F32, name="ident")
        make_identity(nc, ident[:])
        wt = {}
        for k, h in Wd.items():
            t = cons.tile(list(h.shape), F32, name=f"w_{k}")
            nc.sync.dma_start(t[:], h[:])
            wt[k] = t
        st_t = cons.tile([64, MS], F32, name="st_t")
        nc.sync.dma_start(st_t[:], ST[:])
        ecv_t = cons.tile([P, nch_v], F32, name="ecv_t")
        nc.sync.dma_start(ecv_t[:], ec_v[:])
        ece_t = cons.tile([P, nch_e], F32, name="ece_t")
        nc.sync.dma_start(ece_t[:], ec_e[:])
        gvi_t = cons.tile([P, meta["LvP"] // 16], I16, name="gvi_t")
        nc.sync.dma_start(gvi_t[:], gv_idx[:])
        gei_t = cons.tile([P, meta["LeP"] // 16], I16, name="gei_t")
        nc.sync.dma_start(gei_t[:], ge_idx[:])
        dgi_t = cons.tile([P, NT_V], F32, name="dgi_t")
        nc.sync.dma_start(dgi_t[:], deginv_c[:])
        dvi_t = cons.tile([P, NT_V], F32, name="dvi_t")
        nc.sync.dma_start(dvi_t[:], dvinv_c[:])
        cf_t = cons.tile([P, NT_MS], F32, name="cf_t")
        nc.sync.dma_start(cf_t[:], coef_c[:])

        hT = [dram.tile([P, NS], F32, name="hT0"),
              dram.tile([P, NS], F32, name="hT1")]

        def scatter_pass(table, es, used_cols, idx_t, ec_t, cpt, n_tiles,
                         regs, out_dram, out_cols, tag):
            tile_of_chunk = []
            for t, n in enumerate(cpt):
                tile_of_chunk += [t] * n
            n_chunks = len(tile_of_chunk)
            first_c, last_c = {}, {}
            for k, t in enumerate(tile_of_chunk):
                first_c.setdefault(t, k)
                last_c[t] = k
            g_tiles = []
            for call in range(len(regs)):
                if regs[call] == 0:
                    g_tiles.append(None)
                    continue
                gt = sbuf.tile([P, WCH, es], BF16, tag="g", bufs=8,
                               name=f"g{tag}_{call}")
                nc.gpsimd.dma_gather(
                    out_ap=gt[:], in_ap=table[:],
                    idxs_ap=idx_t[:, call * (NI // 16):(call + 1) * (NI // 16)],
                    num_idxs=NI, num_idxs_reg=regs[call], elem_size=es,
                    queue_num=call % GQ)
                g_tiles.append(gt)
            a_cur = [None, -1]
            mega = [None, -1]   # tile, window index
            stg_cur = [None]
            for k in range(n_chunks):
                t = tile_of_chunk[k]
                call, j = k // WCH, k % WCH
                gt = g_tiles[call]
                if gt is None:
                    continue
                w = k // WCH
                if a_cur[1] != w:
                    ab = sbuf.tile([P, WCH, P], BF16, tag="A", bufs=6,
                                   name=f"A{tag}_{w}")
                    nc.vector.tensor_tensor(
                        out=ab[:],
                        in0=ec_t[:, w * WCH:(w + 1) * WCH].to_broadcast(
                            [P, WCH, P]),
                        in1=iota_t[:],
                        op=mybir.AluOpType.is_equal)
                    a_cur = [ab, w]
                mw = t // PSW
                if mega[1] != mw:
                    mega = [psum.tile([P, PSW, 512], F32, tag="ps", bufs=2,
                                      name=f"ps{tag}_{mw}"), mw]
                pt = mega[0]
                q = t % PSW
                nc.tensor.matmul(
                    out=pt[:, q, 0:out_cols],
                    lhsT=a_cur[0][:, j, :],
                    rhs=gt[:, j, 0:used_cols],
                    start=(k == first_c[t]), stop=(k == last_c[t]))
                if k == last_c[t]:
                    if stg_cur[0] is None:
                        stg_cur[0] = sbuf.tile([P, PSW, out_cols],
                                               out_dram.dtype,
                                               tag="pstg", bufs=4,
                                               name=f"pstg{tag}_{t}")
                    nc.vector.tensor_copy(out=stg_cur[0][:, q, :],
                                          in_=pt[:, q, 0:out_cols])
                    if t % PSW == PSW - 1 or t == n_tiles - 1:
                        n_in_w = q + 1
                        rows0 = (t - n_in_w + 1) * P
                        nc.sync.dma_start(
                            out=out_dram[rows0:rows0 + n_in_w * P, :].rearrange(
                                "(j p) c -> p j c", p=P),
                            in_=stg_cur[0][:, 0:n_in_w, :])
                        stg_cur[0] = None

        def elu_u(z_ap, w, cols, tag, i):
            mn = sbuf.tile([P, cols], F32, tag="mn", bufs=2, name=f"mn{tag}{i}")
            nc.vector.tensor_scalar_min(out=mn[:w], in0=z_ap, scalar1=0.0)
            ex = sbuf.tile([P, cols], F32, tag="ex", bufs=2, name=f"ex{tag}{i}")
            nc.scalar.activation(ex[:w], mn[:w], AF.Exp)
            rl = sbuf.tile([P, cols], F32, tag="rl", bufs=2, name=f"rl{tag}{i}")
            nc.scalar.activation(rl[:w], z_ap, AF.Relu)
            u = sbuf.tile([P, cols], F32, tag="u", bufs=2, name=f"u{tag}{i}")
            nc.vector.tensor_add(u[:w], rl[:w], ex[:w])
            return u

        def dense_and_table(l, lhsT_of, table, xinit_dram):
            nh = 1 if l == 0 else 2
            for t in range(NT_V):
                w = min(P, NS - t * P)
                halves = lhsT_of(t, w)
                pf = psum.tile([P, 512], F32, tag="pd", bufs=2, name=f"pf{l}_{t}")
                for hi in range(nh):
                    nc.tensor.matmul(out=pf[:w, 0:257], lhsT=halves[hi],
                                     rhs=wt[f"Wva{l}h{hi}"][:],
                                     start=(hi == 0), stop=(hi == nh - 1))
                F = sbuf.tile([P, 257], F32, tag="F", bufs=2, name=f"F{l}_{t}")
                nc.vector.tensor_add(F[:w], pf[:w, 0:257], wt[f"bva{l}"][:w])
                ew = sbuf.tile([P, 1], F32, tag="ew", bufs=2, name=f"ew{l}_{t}")
                nc.scalar.activation(ew[:w], F[:w, 256:257], AF.Lrelu,
                                     alpha=NEG_SLOPE)
                nc.scalar.activation(ew[:w], ew[:w], AF.Exp)
                stg = sbuf.tile([P, 257], BF16, tag="stg", bufs=2,
                                name=f"stg{l}_{t}")
                nc.vector.tensor_scalar_mul(out=stg[:w, 0:256],
                                            in0=F[:w, 0:256],
                                            scalar1=ew[:w, :])
                nc.vector.tensor_copy(out=stg[:w, 256:257], in_=ew[:w, :])
                nc.sync.dma_start(out=table[t * P:t * P + w, 0:257],
                                  in_=stg[:w, :])
                pi = psum.tile([P, 512], F32, tag="pd", bufs=2, name=f"pi{l}_{t}")
                for hi in range(nh):
                    nc.tensor.matmul(out=pi[:w, 0:256], lhsT=halves[hi],
                                     rhs=wt[f"Wx{l}h{hi}"][:],
                                     start=(hi == 0), stop=(hi == nh - 1))
                xi = sbuf.tile([P, 256], F32, tag="xi", bufs=2, name=f"xi{l}_{t}")
                nc.vector.tensor_add(xi[:w], pi[:w, 0:256], wt[f"bx{l}"][:w])
                nc.sync.dma_start(out=xinit_dram[t * P:t * P + w, :], in_=xi[:w])

        def edge_epilogue(l, rse, yin):
            for t in range(NT_MS):
                w = min(P, MS - t * P)
                rt = sbuf.tile([P, 257], BF16, tag="rt", bufs=2, name=f"rt{l}_{t}")
                nc.sync.dma_start(rt[:w], rse[t * P:t * P + w, :])
                dc = sbuf.tile([P, 1], F32, tag="dc", bufs=2, name=f"dc{l}_{t}")
                nc.vector.tensor_scalar_max(out=dc[:w], in0=rt[:w, 256:257],
                                            scalar1=1e-35)
                di = sbuf.tile([P, 1], F32, tag="di", bufs=2, name=f"di{l}_{t}")
                nc.vector.reciprocal(di[:w], dc[:w])
                z = sbuf.tile([P, 256], F32, tag="z", bufs=2, name=f"z{l}_{t}")
                nc.vector.tensor_scalar_mul(out=z[:w], in0=rt[:w, 0:256],
                                            scalar1=di[:w, :])
                u = elu_u(z[:w], w, 256, f"ee{l}", t)
                uT = []
                for hi in range(2):
                    pT = psum.tile([P, P], F32, tag="pT", bufs=2,
                                   name=f"pT{l}_{t}_{hi}")
                    nc.tensor.transpose(out=pT[:, 0:w],
                                        in_=u[:w, hi * P:(hi + 1) * P],
                                        identity=ident[:w, :w])
                    sT = sbuf.tile([P, P], F32, tag="sT", bufs=2,
                                   name=f"sT{l}_{t}_{hi}")
                    nc.vector.tensor_copy(out=sT[:, 0:w], in_=pT[:, 0:w])
                    uT.append(sT)
                py = psum.tile([P, 512], F32, tag="pd", bufs=2, name=f"py{l}_{t}")
                nc.tensor.matmul(out=py[:w, 0:256],
                                 lhsT=st_t[:, t * P:t * P + w],
                                 rhs=wt[f"Wt{l}bot"][:], start=True, stop=False)
                nc.tensor.matmul(out=py[:w, 0:256], lhsT=uT[0][:, 0:w],
                                 rhs=wt[f"Wt{l}h0"][:], start=False, stop=False)
                nc.tensor.matmul(out=py[:w, 0:256], lhsT=uT[1][:, 0:w],
                                 rhs=wt[f"Wt{l}h1"][:], start=False, stop=True)
                yt = sbuf.tile([P, 256], BF16, tag="yt", bufs=2, name=f"yt{l}_{t}")
                nc.vector.tensor_add(yt[:w], py[:w, 0:256], wt[f"bt{l}"][:w])
                nc.sync.dma_start(out=yin[t * P:t * P + w, :], in_=yt[:w])

        def node_epilogue(l, npart, xinit_dram, h_dram):
            for t in range(NT_V):
                w = min(P, NS - t * P)
                pt = sbuf.tile([P, 256], F32, tag="np", bufs=2, name=f"np{l}_{t}")
                nc.sync.dma_start(pt[:w], npart[t * P:t * P + w, 0:256])
                z = sbuf.tile([P, 256], F32, tag="nz", bufs=2, name=f"nz{l}_{t}")
                nc.vector.tensor_scalar_mul(out=z[:w], in0=pt[:w],
                                            scalar1=dgi_t[:w, t:t + 1])
                u = elu_u(z[:w], w, 256, f"ne{l}", t)
                xi = sbuf.tile([P, 256], F32, tag="nxi", bufs=2,
                               name=f"nxi{l}_{t}")
                nc.sync.dma_start(xi[:w], xinit_dram[t * P:t * P + w, :])
                h = sbuf.tile([P, 256], F32, tag="h", bufs=2, name=f"h{l}_{t}")
                nc.vector.tensor_add(h[:w], u[:w], xi[:w])
                nc.sync.dma_start(out=h_dram[t * P:t * P + w, :], in_=h[:w])
                for hi in range(2):
                    pT = psum.tile([P, P], F32, tag="pT", bufs=2,
                                   name=f"hpT{l}_{t}_{hi}")
                    nc.tensor.transpose(out=pT[:, 0:w],
                                        in_=h[:w, hi * P:(hi + 1) * P],
                                        identity=ident[:w, :w])
                    sT = sbuf.tile([P, P], F32, tag="sT", bufs=2,
                                   name=f"hsT{l}_{t}_{hi}")
                    nc.vector.tensor_copy(out=sT[:, 0:w], in_=pT[:, 0:w])
                    nc.sync.dma_start(out=hT[hi][:, t * P:t * P + w],
                                      in_=sT[:, 0:w])

        rg = [list(range(NCORES))]

        def dphgnn(l, lhsT_of, h_dram):
            table = dram.tile([NS, ESV], BF16, name=f"T{l}")
            xinit = dram.tile([NS, 256], F32, name=f"Xi{l}")
            dense_and_table(l, lhsT_of, table, xinit)
            part = dram.tile([NT_E * P, 257], BF16, name=f"part{l}")
            scatter_pass(table, ESV, 257, gvi_t, ecv_t, meta["cpt_v2e"], NT_E,
                         meta["regs_v"], part, 257, f"v{l}")
            rse = dram.tile([MS, 257], BF16, name=f"rse{l}")
            nc.gpsimd.collective_compute(
                "ReduceScatter", mybir.AluOpType.add, replica_groups=rg,
                ins=[part[0:M, :]], outs=[rse[:]])
            yin = dram.tile([MS, 256], BF16, name=f"yin{l}")
            edge_epilogue(l, rse, yin)
            ytab = dram.tile([M, ESE], BF16, name=f"ytab{l}")
            nc.gpsimd.collective_compute(
                "AllGather", mybir.AluOpType.bypass, replica_groups=rg,
                ins=[yin[:]], outs=[ytab[:]])
            npart = dram.tile([NT_V * P, 256], F32, name=f"npart{l}")
            scatter_pass(ytab, ESE, 256, gei_t, ece_t, meta["cpt_e2v"], NT_V,
                         meta["regs_e"], npart, 256, f"e{l}")
            node_epilogue(l, npart, xinit, h_dram)

        # layer 0
        h0 = dram.tile([NS, 256], F32, name="h0")

        def l0_of(t, w):
            xt = sbuf.tile([P, P], F32, tag="lhs", bufs=4, name=f"xt_{t}")
            nc.sync.dma_start(xt[:, 0:w], XT[:, t * P:t * P + w])
            return [xt[:, 0:w]]

        dphgnn(0, l0_of, h0)

        # layer 1
        h1 = dram.tile([NS, 256], F32, name="h1")

        def l1_of(t, w):
            out = []
            for hi in range(2):
                ht = sbuf.tile([P, P], F32, tag="lhs", bufs=4,
                               name=f"ht_{t}_{hi}")
                nc.sync.dma_start(ht[:, 0:w], hT[hi][:, t * P:t * P + w])
                out.append(ht[:, 0:w])
            return out

        dphgnn(1, l1_of, h1)

        # hyperconv
        table2 = dram.tile([NS, ESH], BF16, name="T2")
        for t in range(NT_V):
            w = min(P, NS - t * P)
            halves = l1_of(t, w)
            pf = psum.tile([P, 512], F32, tag="pd", bufs=2, name=f"pf2_{t}")
            for hi in range(2):
                nc.tensor.matmul(out=pf[:w, 0:128], lhsT=halves[hi],
                                 rhs=wt[f"Wfh{hi}"][:],
                                 start=(hi == 0), stop=(hi == 1))
            xf = sbuf.tile([P, 128], BF16, tag="xf", bufs=2, name=f"xf_{t}")
            nc.vector.tensor_add(xf[:w], pf[:w, 0:128], wt["bf"][:w])
            nc.sync.dma_start(out=table2[t * P:t * P + w, :], in_=xf[:w])
        part3 = dram.tile([NT_E * P, 128], BF16, name="part3")
        scatter_pass(table2, ESH, 128, gvi_t, ecv_t, meta["cpt_v2e"], NT_E,
                     meta["regs_v"], part3, 128, "v2")
        rse3 = dram.tile([MS, 128], BF16, name="rse3")
        nc.gpsimd.collective_compute(
            "ReduceScatter", mybir.AluOpType.add, replica_groups=rg,
            ins=[part3[0:M, :]], outs=[rse3[:]])
        yin3 = dram.tile([MS, 128], BF16, name="yin3")
        for t in range(NT_MS):
            w = min(P, MS - t * P)
            rt = sbuf.tile([P, 128], BF16, tag="rt3", bufs=2, name=f"rt3_{t}")
            nc.sync.dma_start(rt[:w], rse3[t * P:t * P + w, :])
            yt = sbuf.tile([P, 128], BF16, tag="yt3", bufs=2, name=f"yt3_{t}")
            nc.vector.tensor_scalar_mul(out=yt[:w], in0=rt[:w],
                                        scalar1=cf_t[:w, t:t + 1])
            nc.sync.dma_start(out=yin3[t * P:t * P + w, :], in_=yt[:w])
        ytab3 = dram.tile([M, 128], BF16, name="ytab3")
        nc.gpsimd.collective_compute(
            "AllGather", mybir.AluOpType.bypass, replica_groups=rg,
            ins=[yin3[:]], outs=[ytab3[:]])
        npart3 = dram.tile([NT_V * P, 128], F32, name="npart3")
        scatter_pass(ytab3, 128, 128, gei_t, ece_t, meta["cpt_e2v"], NT_V,
                     meta["regs_e"], npart3, 128, "e2")
        for t in range(NT_V):
            w = min(P, NS - t * P)
            pt = sbuf.tile([P, 128], F32, tag="fp", bufs=2, name=f"fp_{t}")
            nc.sync.dma_start(pt[:w], npart3[t * P:t * P + w, :])
            ot = sbuf.tile([P, 128], F32, tag="fo", bufs=2, name=f"fo_{t}")
            nc.vector.tensor_scalar_mul(out=ot[:w], in0=pt[:w],
                                        scalar1=dvi_t[:w, t:t + 1])
            nc.sync.dma_start(out=yout[t * P:t * P + w, :], in_=ot[:w])
        ctx.close()

    nc.compile()
    return nc


_CACHED = {}


def kernel(**inputs):
    in_maps, meta = _prep(inputs)
    key = (meta["LvP"], meta["LeP"], tuple(meta["cpt_v2e"]),
           tuple(meta["cpt_e2v"]))
    if key not in _CACHED:
        _CACHED[key] = build_program(meta)
    nc = _CACHED[key]
    res = run_bass_kernel_spmd(nc, in_maps, list(range(NCORES)))
    out = np.concatenate([res.results[c]["yout"] for c in range(NCORES)],
                         axis=0)
    return np.ascontiguousarray(out.astype(np.float32))
